# revision 20
# baseline (speedup 1.0000x reference)
"""Trainium2 Bass kernel for the GaussianImageModel problem.

Computes img = clip(num/(den+eps)) where
  num[a,b,c] = sum_k w[a,b,k] * sigmoid(color_logits)[k,c]
  den[a,b]   = sum_k w[a,b,k]
  w[a,b,k]   = softplus(log_amp)[k] * exp(-0.5 * q[a,b,k])
  q          = vx^2/(sx^2+eps) + vy^2/(sy^2+eps)   (rotated offsets)

-0.5*q + log(amp) is a quadratic polynomial in the pixel coords (x, y):
  poly = a0*x^2 + a1*y^2 + a2*x*y + a3*x + a4*y + a5        (per Gaussian)

Two device paths, selected on the host from the actual input values:

* FAST path: when the grid is a separable meshgrid (x depends only on the
  row index, y only on the column index) and the cross coefficient a2 is
  exactly zero for every Gaussian (true whenever sx==sy, i.e. isotropic
  Gaussians -- the case produced by setup_inputs), the weight factorizes
  w[a,b,k] = U[a,k]*V[b,k].  U/V are computed on-device via a tiny
  3-term matmul + exp, and the pixel reduction over k becomes a single
  K-contraction matmul.  ~300K exps instead of 67M.

* GENERAL path: arbitrary grid / anisotropic / rotated Gaussians.
  (HW,6) pixel-basis @ (6,K) coeffs -> exp -> (HW,K) @ (K,4) with both
  matmuls on the tensor engine and exp on the scalar engine.

Work is sharded over 8 NeuronCores by pixel rows (data-parallel over
pixels; the (K,.) parameters are replicated), matching the data-parallel
sharding hint.
"""

import math

import numpy as np

import concourse.bass as bass
import concourse.bacc as bacc
import concourse.mybir as mybir
from concourse.bass_utils import run_bass_kernel_spmd
from concourse.tile import TileContext

F32 = mybir.dt.float32
F32R = mybir.dt.float32r
AF = mybir.ActivationFunctionType
ALU = mybir.AluOpType

H, W, K = 512, 512, 256
NCORES = 8
ROWS = H // NCORES  # 64 pixel rows per core
EPS = 1e-6

# Set by kernel() when BASS_TRACE=1: exec time in ns of the slowest core.
LAST_EXEC_NS = None


# --------------------------------------------------------------------------
# FAST path bass module
# --------------------------------------------------------------------------
P3W = ROWS + W + K + K  # packed [xb | yb | pu | pv] along free dim


def _build_fast():
    nc = bacc.Bacc()
    p3 = nc.dram_tensor("p3", [3, P3W], F32, kind="ExternalInput")
    cc = nc.dram_tensor("cc", [128, 8], F32, kind="ExternalInput")
    out = nc.dram_tensor("out", [3, ROWS, W], F32, kind="ExternalOutput")

    with TileContext(nc) as tc:
        with (
            tc.tile_pool(name="sb", bufs=1) as sb,
            tc.tile_pool(name="ps", bufs=1, space="PSUM") as ps,
        ):
            p3_t = sb.tile([3, P3W], F32, tag="p3")
            cc_t = sb.tile([128, 8], F32, tag="cc")
            nc.sync.dma_start(p3_t[:], p3[:, :])
            nc.sync.dma_start(cc_t[:], cc[:, :])
            xb_t = p3_t[:, 0:ROWS]
            yb_t = p3_t[:, ROWS : ROWS + W]
            pu_t = p3_t[:, ROWS + W : ROWS + W + K]
            pv_t = p3_t[:, ROWS + W + K : ROWS + W + 2 * K]

            # Warm the PE (HAM un-throttle needs ~3.4us of sustained array
            # activity) with dummy bf16 matmuls on scratch data while the
            # input DMA is in flight; the real matmuls then run at 2.4 GHz.
            ps_u = [ps.tile([128, ROWS], F32, tag=f"psu{i}", name=f"psu{i}") for i in range(2)]
            ps_v = [ps.tile([128, W], F32, tag=f"psv{i}", name=f"psv{i}") for i in range(2)]
            ps_o = [ps.tile([ROWS, W], F32, tag=f"pso{c}", name=f"pso{c}") for c in range(4)]
            warm_sb = sb.tile([128, 512], mybir.dt.bfloat16, tag="warm")
            nc.gpsimd.memset(warm_sb[:], 0.0)
            for i in range(8):
                nc.tensor.matmul(
                    ps_o[0][:], warm_sb[:, 0:64], warm_sb[:], start=True, stop=True
                )
            v_sb = sb.tile([128, 2 * W], F32R, tag="v")
            for kc in range(2):
                ksl = bass.ts(kc, 128)
                nc.tensor.matmul(ps_v[kc][:], pv_t[:, ksl], yb_t, start=True, stop=True)
            for kc in range(2):
                ksl = bass.ts(kc, 128)
                nc.tensor.matmul(ps_u[kc][:], pu_t[:, ksl], xb_t, start=True, stop=True)
            for kc in range(2):
                nc.scalar.activation(v_sb[:, bass.ts(kc, W)], ps_v[kc][:], AF.Exp)

            # keep the PE-HAM warm across the exp-wait gap
            for i in range(2):
                nc.tensor.matmul(
                    ps_o[0][:], warm_sb[:, 0:64], warm_sb[:], start=True, stop=True
                )

            # T[k, (kc,c,a)] = exp(logU[k,a] + log cc[k,c])  (bias per k)
            t_sb = sb.tile([128, 8 * ROWS], F32R, tag="t")
            for c in (3, 0, 1, 2):  # den first so the epilogue overlaps PE
                for kc in range(2):
                    j = kc * 4 + c
                    nc.scalar.activation(
                        t_sb[:, bass.ts(j, ROWS)],
                        ps_u[kc][:],
                        AF.Exp,
                        bias=cc_t[:, j : j + 1],
                    )

            # out[(c), a, b] = sum_k T[k, c*ROWS+a] * V[k, b]; den (c=3) first
            for c in (3, 0, 1, 2):
                for kc in range(2):
                    nc.tensor.matmul(
                        ps_o[c][:],
                        t_sb[:, bass.ts(kc * 4 + c, ROWS)],
                        v_sb[:, bass.ts(kc, W)],
                        start=(kc == 0),
                        stop=(kc == 1),
                    )

            # epilogue: res = clip(num * 1/(den+eps), 0, 1), pipelined per
            # channel against the remaining PE matmuls.
            den_sb = sb.tile([ROWS, W], F32, tag="den")
            nc.scalar.activation(den_sb[:], ps_o[3][:], AF.Copy, bias=EPS)
            rec_sb = sb.tile([ROWS, W], F32, tag="rec")
            scr_sb = sb.tile([ROWS, W], F32, tag="scr")
            nc.vector.reciprocal_approx_accurate(rec_sb[:], den_sb[:], scr_sb[:])
            res_sb = sb.tile([ROWS, 3 * W], F32, tag="res")
            for c in range(3):
                nc.vector.tensor_tensor(
                    res_sb[:, bass.ts(c, W)], ps_o[c][:], rec_sb[:], ALU.mult
                )
                nc.vector.tensor_scalar(
                    res_sb[:, bass.ts(c, W)], res_sb[:, bass.ts(c, W)],
                    0.0, 1.0, ALU.max, ALU.min,
                )
                eng = nc.sync if c != 1 else nc.scalar
                eng.dma_start(out[c, :, :], res_sb[:, bass.ts(c, W)])
    nc.compile()
    return nc


# --------------------------------------------------------------------------
# GENERAL path bass module
# --------------------------------------------------------------------------
PIX = ROWS * W  # pixels per core
TILE = 512      # pixels per inner tile
NT = PIX // TILE


def _build_general():
    nc = bacc.Bacc()
    pb = nc.dram_tensor("pb", [6, PIX], F32, kind="ExternalInput")
    m6 = nc.dram_tensor("m6", [6, K], F32, kind="ExternalInput")
    cc = nc.dram_tensor("cc", [128, 8], F32, kind="ExternalInput")
    nd = nc.dram_tensor("nd", [4, PIX], F32, kind="ExternalOutput")

    with TileContext(nc) as tc:
        with (
            tc.tile_pool(name="sb", bufs=1) as sb,
            tc.tile_pool(name="work", bufs=3) as work,
            tc.tile_pool(name="ps", bufs=3, space="PSUM") as psg,
            tc.tile_pool(name="pso", bufs=2, space="PSUM") as pso,
        ):
            GRP = 8
            pb_t = sb.tile([6, PIX], F32, tag="pb")
            m6_t = sb.tile([6, K], F32, tag="m6")
            cc_t = sb.tile([128, 8], F32, tag="cc")
            nc.gpsimd.dma_start(pb_t[:], pb[:, :])
            nc.gpsimd.dma_start(m6_t[:], m6[:, :])
            nc.gpsimd.dma_start(cc_t[:], cc[:, :])

            for g in range(NT // GRP):
                nd_g = work.tile([4, GRP * TILE], F32, tag="ndg", name=f"ndg{g}")
                for tt in range(GRP):
                    t = g * GRP + tt
                    psl = bass.ts(t, TILE)
                    g_ps = psg.tile([128, 2 * TILE], F32, tag="g", name=f"g{t}")
                    g_sb = work.tile([128, 2 * TILE], F32, tag="gsb", name=f"gsb{t}")
                    for kc in range(2):
                        nc.tensor.matmul(
                            g_ps[:, bass.ts(kc, TILE)],
                            m6_t[:, bass.ts(kc, 128)],
                            pb_t[:, psl],
                            start=True,
                            stop=True,
                        )
                    nc.scalar.activation(g_sb[:], g_ps[:], AF.Exp)
                    o_ps = pso.tile([4, TILE], F32, tag="o", name=f"o{t}")
                    for kc in range(2):
                        nc.tensor.matmul(
                            o_ps[:],
                            cc_t[:, bass.ts(kc, 4)],
                            g_sb[:, bass.ts(kc, TILE)],
                            start=(kc == 0),
                            stop=(kc == 1),
                        )
                    nc.vector.tensor_copy(nd_g[:, bass.ts(tt, TILE)], o_ps[:])
                nc.sync.dma_start(
                    nd[:, g * GRP * TILE : (g + 1) * GRP * TILE], nd_g[:]
                )
    nc.compile()
    return nc


# --------------------------------------------------------------------------
# host-side parameter math
# --------------------------------------------------------------------------
def _poly_coeffs(mu, log_scales, theta, log_amp):
    """Per-Gaussian coefficients of -0.5*q + log(amp), float64.

    Returns (A, B, C, a0..a5) where q = A dx^2 + B dy^2 + C dx dy.
    """
    sc = np.exp(log_scales.astype(np.float64))
    ia = 1.0 / (sc[:, 0] ** 2 + EPS)
    ib = 1.0 / (sc[:, 1] ** 2 + EPS)
    c = np.cos(theta.astype(np.float64))
    s = np.sin(theta.astype(np.float64))
    A = c * c * ia + s * s * ib
    B = s * s * ia + c * c * ib
    C = 2.0 * c * s * (ia - ib)
    mx = mu[:, 0].astype(np.float64)
    my = mu[:, 1].astype(np.float64)
    lamp = np.log(np.logaddexp(0.0, log_amp.astype(np.float64)[:, 0]))
    a0 = -0.5 * A
    a1 = -0.5 * B
    a2 = -0.5 * C
    a3 = A * mx + 0.5 * C * my
    a4 = B * my + 0.5 * C * mx
    a5 = -0.5 * (A * mx * mx + B * my * my + C * mx * my) + lamp
    return A, B, C, a0, a1, a2, a3, a4, a5


def _cc_table(color_logits, log=False):
    """(128, 8) table: cc[p, kc*4+c] = [sigmoid(colors) | 1][kc*128+p, c].

    log=True returns elementwise log (used as exp bias on the fast path);
    log(sigmoid(x)) = -softplus(-x), log(1) = 0.
    """
    cl = color_logits.astype(np.float64)
    if log:
        col = -np.logaddexp(0.0, -cl)
        cc4 = np.concatenate([col, np.zeros((K, 1))], axis=1).astype(np.float32)
    else:
        col = 1.0 / (1.0 + np.exp(-cl))
        cc4 = np.concatenate([col, np.ones((K, 1))], axis=1).astype(np.float32)
    return np.ascontiguousarray(
        cc4.reshape(2, 128, 4).transpose(1, 0, 2).reshape(128, 8)
    )


_FAST_NC = None
_GEN_NC = None


def kernel(grid, mu, log_scales, theta, color_logits, log_amp):
    global _FAST_NC, _GEN_NC, LAST_EXEC_NS
    grid = np.ascontiguousarray(grid, dtype=np.float32)
    assert grid.shape == (H, W, 2)
    assert mu.shape == (K, 2) and theta.shape == (K,)

    A, B, C, a0, a1, a2, a3, a4, a5 = _poly_coeffs(mu, log_scales, theta, log_amp)

    xs = grid[:, 0, 0]
    ys = grid[0, :, 1]
    separable = np.array_equal(
        grid[:, :, 0], np.broadcast_to(xs[:, None], (H, W))
    ) and np.array_equal(grid[:, :, 1], np.broadcast_to(ys[None, :], (H, W)))
    fast_ok = separable and float(np.abs(C).max()) == 0.0

    core_ids = list(range(NCORES))
    if fast_ok:
        cc_log = _cc_table(color_logits, log=True)
        pu = np.stack([a0, a3, a5]).astype(np.float32)  # (3, K) poly in x
        pv = np.stack([a1, a4, np.zeros(K)]).astype(np.float32)  # (3, K) in y
        yb = np.stack([ys * ys, ys, np.ones(W, np.float32)]).astype(np.float32)
        in_maps = []
        for i in core_ids:
            xsl = xs[i * ROWS : (i + 1) * ROWS].astype(np.float32)
            xbl = np.stack([xsl * xsl, xsl, np.ones(ROWS, np.float32)])
            p3 = np.concatenate([xbl, yb, pu, pv], axis=1)
            in_maps.append(
                {
                    "p3": np.ascontiguousarray(p3, dtype=np.float32),
                    "cc": cc_log,
                }
            )
        if _FAST_NC is None:
            _FAST_NC = _build_fast()
        r = run_bass_kernel_spmd(_FAST_NC, in_maps, core_ids)
        LAST_EXEC_NS = r.exec_time_ns
        slabs = [r.results[i]["out"].transpose(1, 2, 0) for i in core_ids]
        return np.ascontiguousarray(np.concatenate(slabs, axis=0))

    # general path: (HW,6) basis, full quadratic
    x = grid[:, :, 0].astype(np.float32).reshape(H * W)
    y = grid[:, :, 1].astype(np.float32).reshape(H * W)
    pbasis = np.stack([x * x, y * y, x * y, x, y, np.ones(H * W, np.float32)])
    m6 = np.stack([a0, a1, a2, a3, a4, a5]).astype(np.float32)  # (6, K)
    cc = _cc_table(color_logits)
    in_maps = []
    for i in core_ids:
        in_maps.append(
            {
                "pb": np.ascontiguousarray(pbasis[:, i * PIX : (i + 1) * PIX]),
                "m6": m6,
                "cc": cc,
            }
        )
    if _GEN_NC is None:
        _GEN_NC = _build_general()
    r = run_bass_kernel_spmd(_GEN_NC, in_maps, core_ids)
    LAST_EXEC_NS = r.exec_time_ns
    parts = []
    for i in core_ids:
        nd = r.results[i]["nd"]  # (4, PIX)
        img = np.clip(nd[:3] / (nd[3] + EPS), 0.0, 1.0)  # (3, PIX)
        parts.append(img.T.reshape(ROWS, W, 3))
    return np.ascontiguousarray(np.concatenate(parts, axis=0), dtype=np.float32)


# revision 21
# speedup vs baseline: 1.0670x; 1.0670x over previous
"""Trainium2 Bass kernel for the GaussianImageModel problem.

Computes img = clip(num/(den+eps)) where
  num[a,b,c] = sum_k w[a,b,k] * sigmoid(color_logits)[k,c]
  den[a,b]   = sum_k w[a,b,k]
  w[a,b,k]   = softplus(log_amp)[k] * exp(-0.5 * q[a,b,k])
  q          = vx^2/(sx^2+eps) + vy^2/(sy^2+eps)   (rotated offsets)

-0.5*q + log(amp) is a quadratic polynomial in the pixel coords (x, y):
  poly = a0*x^2 + a1*y^2 + a2*x*y + a3*x + a4*y + a5        (per Gaussian)

Two device paths, selected on the host from the actual input values:

* FAST path: when the grid is a separable meshgrid (x depends only on the
  row index, y only on the column index) and the cross coefficient a2 is
  exactly zero for every Gaussian (true whenever sx==sy, i.e. isotropic
  Gaussians -- the case produced by setup_inputs), the weight factorizes
  w[a,b,k] = U[a,k]*V[b,k].  U/V are computed on-device via a tiny
  3-term matmul + exp, and the pixel reduction over k becomes a single
  K-contraction matmul.  ~300K exps instead of 67M.

* GENERAL path: arbitrary grid / anisotropic / rotated Gaussians.
  (HW,6) pixel-basis @ (6,K) coeffs -> exp -> (HW,K) @ (K,4) with both
  matmuls on the tensor engine and exp on the scalar engine.

Work is sharded over 8 NeuronCores by pixel rows (data-parallel over
pixels; the (K,.) parameters are replicated), matching the data-parallel
sharding hint.
"""

import math

import numpy as np

import concourse.bass as bass
import concourse.bacc as bacc
import concourse.mybir as mybir
from concourse.bass_utils import run_bass_kernel_spmd
from concourse.tile import TileContext

F32 = mybir.dt.float32
F32R = mybir.dt.float32r
AF = mybir.ActivationFunctionType
ALU = mybir.AluOpType

H, W, K = 512, 512, 256
NCORES = 8
ROWS = H // NCORES  # 64 pixel rows per core
EPS = 1e-6

# Set by kernel() when BASS_TRACE=1: exec time in ns of the slowest core.
LAST_EXEC_NS = None


# --------------------------------------------------------------------------
# FAST path bass module
# --------------------------------------------------------------------------
P3W = ROWS + W + K + K  # packed [xb | yb | pu | pv] along free dim


def _build_fast():
    nc = bacc.Bacc()
    p3 = nc.dram_tensor("p3", [3, P3W], F32, kind="ExternalInput")
    cc = nc.dram_tensor("cc", [128, 8], F32, kind="ExternalInput")
    out = nc.dram_tensor("out", [3, ROWS, W], F32, kind="ExternalOutput")

    with TileContext(nc) as tc:
        with (
            tc.tile_pool(name="sb", bufs=1) as sb,
            tc.tile_pool(name="ps", bufs=1, space="PSUM") as ps,
        ):
            p3_t = sb.tile([3, P3W], F32, tag="p3")
            cc_t = sb.tile([128, 8], F32, tag="cc")
            nc.sync.dma_start(p3_t[:], p3[:, :])
            nc.sync.dma_start(cc_t[:], cc[:, :])
            xb_t = p3_t[:, 0:ROWS]
            yb_t = p3_t[:, ROWS : ROWS + W]
            pu_t = p3_t[:, ROWS + W : ROWS + W + K]
            pv_t = p3_t[:, ROWS + W + K : ROWS + W + 2 * K]

            # log-U / log-V via 3-term basis matmuls on PE.
            ps_u = [ps.tile([128, ROWS], F32, tag=f"psu{i}", name=f"psu{i}") for i in range(2)]
            ps_v = [ps.tile([128, W], F32, tag=f"psv{i}", name=f"psv{i}") for i in range(2)]
            v_sb = sb.tile([128, 2 * W], F32R, tag="v")
            for kc in range(2):
                ksl = bass.ts(kc, 128)
                nc.tensor.matmul(ps_v[kc][:], pv_t[:, ksl], yb_t, start=True, stop=True)
            for kc in range(2):
                ksl = bass.ts(kc, 128)
                nc.tensor.matmul(ps_u[kc][:], pu_t[:, ksl], xb_t, start=True, stop=True)
            for kc in range(2):
                nc.scalar.activation(v_sb[:, bass.ts(kc, W)], ps_v[kc][:], AF.Exp)

            # T[k, (kc,c,a)] = exp(logU[k,a] + log cc[k,c])  (bias per k)
            t_sb = sb.tile([128, 8 * ROWS], F32R, tag="t")
            for c in (3, 0, 1, 2):  # den first so the epilogue overlaps PE
                for kc in range(2):
                    j = kc * 4 + c
                    nc.scalar.activation(
                        t_sb[:, bass.ts(j, ROWS)],
                        ps_u[kc][:],
                        AF.Exp,
                        bias=cc_t[:, j : j + 1],
                    )

            # out[(c), a, b] = sum_k T[k, c*ROWS+a] * V[k, b]; den (c=3) first
            ps_o = [ps.tile([ROWS, W], F32, tag=f"pso{c}", name=f"pso{c}") for c in range(4)]
            for c in (3, 0, 1, 2):
                for kc in range(2):
                    nc.tensor.matmul(
                        ps_o[c][:],
                        t_sb[:, bass.ts(kc * 4 + c, ROWS)],
                        v_sb[:, bass.ts(kc, W)],
                        start=(kc == 0),
                        stop=(kc == 1),
                    )

            # epilogue: res = clip(num * 1/(den+eps), 0, 1), pipelined per
            # channel against the remaining PE matmuls.
            den_sb = sb.tile([ROWS, W], F32, tag="den")
            nc.scalar.activation(den_sb[:], ps_o[3][:], AF.Copy, bias=EPS)
            rec_sb = sb.tile([ROWS, W], F32, tag="rec")
            scr_sb = sb.tile([ROWS, W], F32, tag="scr")
            nc.vector.reciprocal_approx_accurate(rec_sb[:], den_sb[:], scr_sb[:])
            res_sb = sb.tile([ROWS, 3 * W], F32, tag="res")
            for c in range(3):
                nc.vector.tensor_tensor(
                    res_sb[:, bass.ts(c, W)], ps_o[c][:], rec_sb[:], ALU.mult
                )
                nc.vector.tensor_scalar(
                    res_sb[:, bass.ts(c, W)], res_sb[:, bass.ts(c, W)],
                    0.0, 1.0, ALU.max, ALU.min,
                )
                eng = nc.sync if c != 1 else nc.scalar
                eng.dma_start(out[c, :, :], res_sb[:, bass.ts(c, W)])
    nc.compile()
    return nc


# --------------------------------------------------------------------------
# GENERAL path bass module
# --------------------------------------------------------------------------
PIX = ROWS * W  # pixels per core
TILE = 512      # pixels per inner tile
NT = PIX // TILE


def _build_general():
    nc = bacc.Bacc()
    pb = nc.dram_tensor("pb", [6, PIX], F32, kind="ExternalInput")
    m6 = nc.dram_tensor("m6", [6, K], F32, kind="ExternalInput")
    cc = nc.dram_tensor("cc", [128, 8], F32, kind="ExternalInput")
    nd = nc.dram_tensor("nd", [4, PIX], F32, kind="ExternalOutput")

    with TileContext(nc) as tc:
        with (
            tc.tile_pool(name="sb", bufs=1) as sb,
            tc.tile_pool(name="work", bufs=3) as work,
            tc.tile_pool(name="ps", bufs=3, space="PSUM") as psg,
            tc.tile_pool(name="pso", bufs=2, space="PSUM") as pso,
        ):
            GRP = 8
            pb_t = sb.tile([6, PIX], F32, tag="pb")
            m6_t = sb.tile([6, K], F32, tag="m6")
            cc_t = sb.tile([128, 8], F32, tag="cc")
            nc.gpsimd.dma_start(pb_t[:], pb[:, :])
            nc.gpsimd.dma_start(m6_t[:], m6[:, :])
            nc.gpsimd.dma_start(cc_t[:], cc[:, :])

            for g in range(NT // GRP):
                nd_g = work.tile([4, GRP * TILE], F32, tag="ndg", name=f"ndg{g}")
                for tt in range(GRP):
                    t = g * GRP + tt
                    psl = bass.ts(t, TILE)
                    g_ps = psg.tile([128, 2 * TILE], F32, tag="g", name=f"g{t}")
                    g_sb = work.tile([128, 2 * TILE], F32, tag="gsb", name=f"gsb{t}")
                    for kc in range(2):
                        nc.tensor.matmul(
                            g_ps[:, bass.ts(kc, TILE)],
                            m6_t[:, bass.ts(kc, 128)],
                            pb_t[:, psl],
                            start=True,
                            stop=True,
                        )
                    nc.scalar.activation(g_sb[:], g_ps[:], AF.Exp)
                    o_ps = pso.tile([4, TILE], F32, tag="o", name=f"o{t}")
                    for kc in range(2):
                        nc.tensor.matmul(
                            o_ps[:],
                            cc_t[:, bass.ts(kc, 4)],
                            g_sb[:, bass.ts(kc, TILE)],
                            start=(kc == 0),
                            stop=(kc == 1),
                        )
                    nc.vector.tensor_copy(nd_g[:, bass.ts(tt, TILE)], o_ps[:])
                nc.sync.dma_start(
                    nd[:, g * GRP * TILE : (g + 1) * GRP * TILE], nd_g[:]
                )
    nc.compile()
    return nc


# --------------------------------------------------------------------------
# host-side parameter math
# --------------------------------------------------------------------------
def _poly_coeffs(mu, log_scales, theta, log_amp):
    """Per-Gaussian coefficients of -0.5*q + log(amp), float64.

    Returns (A, B, C, a0..a5) where q = A dx^2 + B dy^2 + C dx dy.
    """
    sc = np.exp(log_scales.astype(np.float64))
    ia = 1.0 / (sc[:, 0] ** 2 + EPS)
    ib = 1.0 / (sc[:, 1] ** 2 + EPS)
    c = np.cos(theta.astype(np.float64))
    s = np.sin(theta.astype(np.float64))
    A = c * c * ia + s * s * ib
    B = s * s * ia + c * c * ib
    C = 2.0 * c * s * (ia - ib)
    mx = mu[:, 0].astype(np.float64)
    my = mu[:, 1].astype(np.float64)
    lamp = np.log(np.logaddexp(0.0, log_amp.astype(np.float64)[:, 0]))
    a0 = -0.5 * A
    a1 = -0.5 * B
    a2 = -0.5 * C
    a3 = A * mx + 0.5 * C * my
    a4 = B * my + 0.5 * C * mx
    a5 = -0.5 * (A * mx * mx + B * my * my + C * mx * my) + lamp
    return A, B, C, a0, a1, a2, a3, a4, a5


def _cc_table(color_logits, log=False):
    """(128, 8) table: cc[p, kc*4+c] = [sigmoid(colors) | 1][kc*128+p, c].

    log=True returns elementwise log (used as exp bias on the fast path);
    log(sigmoid(x)) = -softplus(-x), log(1) = 0.
    """
    cl = color_logits.astype(np.float64)
    if log:
        col = -np.logaddexp(0.0, -cl)
        cc4 = np.concatenate([col, np.zeros((K, 1))], axis=1).astype(np.float32)
    else:
        col = 1.0 / (1.0 + np.exp(-cl))
        cc4 = np.concatenate([col, np.ones((K, 1))], axis=1).astype(np.float32)
    return np.ascontiguousarray(
        cc4.reshape(2, 128, 4).transpose(1, 0, 2).reshape(128, 8)
    )


_FAST_NC = None
_GEN_NC = None


def kernel(grid, mu, log_scales, theta, color_logits, log_amp):
    global _FAST_NC, _GEN_NC, LAST_EXEC_NS
    grid = np.ascontiguousarray(grid, dtype=np.float32)
    assert grid.shape == (H, W, 2)
    assert mu.shape == (K, 2) and theta.shape == (K,)

    A, B, C, a0, a1, a2, a3, a4, a5 = _poly_coeffs(mu, log_scales, theta, log_amp)

    xs = grid[:, 0, 0]
    ys = grid[0, :, 1]
    separable = np.array_equal(
        grid[:, :, 0], np.broadcast_to(xs[:, None], (H, W))
    ) and np.array_equal(grid[:, :, 1], np.broadcast_to(ys[None, :], (H, W)))
    fast_ok = separable and float(np.abs(C).max()) == 0.0

    core_ids = list(range(NCORES))
    if fast_ok:
        cc_log = _cc_table(color_logits, log=True)
        pu = np.stack([a0, a3, a5]).astype(np.float32)  # (3, K) poly in x
        pv = np.stack([a1, a4, np.zeros(K)]).astype(np.float32)  # (3, K) in y
        yb = np.stack([ys * ys, ys, np.ones(W, np.float32)]).astype(np.float32)
        in_maps = []
        for i in core_ids:
            xsl = xs[i * ROWS : (i + 1) * ROWS].astype(np.float32)
            xbl = np.stack([xsl * xsl, xsl, np.ones(ROWS, np.float32)])
            p3 = np.concatenate([xbl, yb, pu, pv], axis=1)
            in_maps.append(
                {
                    "p3": np.ascontiguousarray(p3, dtype=np.float32),
                    "cc": cc_log,
                }
            )
        if _FAST_NC is None:
            _FAST_NC = _build_fast()
        r = run_bass_kernel_spmd(_FAST_NC, in_maps, core_ids)
        LAST_EXEC_NS = r.exec_time_ns
        slabs = [r.results[i]["out"].transpose(1, 2, 0) for i in core_ids]
        return np.ascontiguousarray(np.concatenate(slabs, axis=0))

    # general path: (HW,6) basis, full quadratic
    x = grid[:, :, 0].astype(np.float32).reshape(H * W)
    y = grid[:, :, 1].astype(np.float32).reshape(H * W)
    pbasis = np.stack([x * x, y * y, x * y, x, y, np.ones(H * W, np.float32)])
    m6 = np.stack([a0, a1, a2, a3, a4, a5]).astype(np.float32)  # (6, K)
    cc = _cc_table(color_logits)
    in_maps = []
    for i in core_ids:
        in_maps.append(
            {
                "pb": np.ascontiguousarray(pbasis[:, i * PIX : (i + 1) * PIX]),
                "m6": m6,
                "cc": cc,
            }
        )
    if _GEN_NC is None:
        _GEN_NC = _build_general()
    r = run_bass_kernel_spmd(_GEN_NC, in_maps, core_ids)
    LAST_EXEC_NS = r.exec_time_ns
    parts = []
    for i in core_ids:
        nd = r.results[i]["nd"]  # (4, PIX)
        img = np.clip(nd[:3] / (nd[3] + EPS), 0.0, 1.0)  # (3, PIX)
        parts.append(img.T.reshape(ROWS, W, 3))
    return np.ascontiguousarray(np.concatenate(parts, axis=0), dtype=np.float32)
